# revision 58
# baseline (speedup 1.0000x reference)
"""Trainium2 Bass kernel for an ALBERT-style seq2seq block (self-attn + cross-attn).

Sharding: 8 cores = (batch b in 0..3) x (decoder-row half in 0..1).
Each core computes its 512 decoder rows of the final output for its batch;
k/v/ek/ev projections are duplicated across the 2 cores sharing a batch
(zero inter-core communication).

On-chip layout is feature-major: activations live as [hidden, token] so every
matmul contracts over the partition axis. Softmax denominators come from a
fused [v_h | ones] stationary block (bf16 PV matmul, rows 64:128 = denom).
Phases are interleaved to keep the PE dense: encoder-k projection is emitted
inside the self-attention pair loop, encoder-v inside the out-proj/LN phase.
"""

import sys

sys.path.insert(0, "/opt/trn_rl_repo")

import numpy as np

import concourse.bacc as bacc
import concourse.mybir as mybir
from concourse.bass_utils import run_bass_kernel_spmd
from concourse.masks import make_identity
from concourse.tile import TileContext

F32 = mybir.dt.float32
F32R = mybir.dt.float32r
BF16 = mybir.dt.bfloat16
AF = mybir.ActivationFunctionType
ALU = mybir.AluOpType

P = 128          # partitions
H = 1024         # hidden
NT = H // P      # 8 tiles over hidden
NH = 16          # heads
D = 64           # head dim
T = 1024         # sequence length (encoder and decoder)
R = 512          # decoder rows per core
B = 4
EPS = 1e-12


def build_kernel():
    nc = bacc.Bacc("TRN2", num_devices=8)

    decT = nc.declare_dram_parameter("decT", [P, NT, T], F32, isOutput=False)
    dqT_d = nc.declare_dram_parameter("decqT", [P, NT, R], F32, isOutput=False)
    encT = nc.declare_dram_parameter("encT", [P, NT, T], F32, isOutput=False)
    wqT = nc.declare_dram_parameter("wqT", [NT, P, NT, P], F32, isOutput=False)
    wkT = nc.declare_dram_parameter("wkT", [NT, P, NT, P], F32, isOutput=False)
    wvT = nc.declare_dram_parameter("wvT", [NT, P, NT, P], F32, isOutput=False)
    wdT = nc.declare_dram_parameter("wdT", [NT, P, NT, P], F32, isOutput=False)
    bq_d = nc.declare_dram_parameter("bq", [P, NT, 1], F32, isOutput=False)
    bk_d = nc.declare_dram_parameter("bk", [P, NT, 1], F32, isOutput=False)
    bv_d = nc.declare_dram_parameter("bv", [P, NT, 1], F32, isOutput=False)
    bd_d = nc.declare_dram_parameter("bd", [P, NT, 1], F32, isOutput=False)
    lng_d = nc.declare_dram_parameter("lng", [P, NT, 1], F32, isOutput=False)
    lnb_d = nc.declare_dram_parameter("lnb", [P, NT, 1], F32, isOutput=False)
    mt_d = nc.declare_dram_parameter("mt", [P, NT, 1], F32, isOutput=False)
    ms_d = nc.declare_dram_parameter("ms", [P, NT, 1], F32, isOutput=False)
    ones_d = nc.declare_dram_parameter("onesc", [P, NH * D], BF16, isOutput=False)
    onesf_d = nc.declare_dram_parameter("onesf", [P, 1], F32, isOutput=False)
    onesr_d = nc.declare_dram_parameter("onesr", [1, P], F32, isOutput=False)
    out_d = nc.declare_dram_parameter("out", [P, NT, R], F32, isOutput=True)

    def col_view(d):  # pre-tiled [P, NT, 1]
        return d.ap()

    def wchunk_view(d, ot):  # pre-tiled weights [NT, P, NT, P] -> [P, NT, P]
        return d.ap()[ot].bitcast(F32R)

    with TileContext(nc) as tc:
        with tc.tile_pool(name="base", bufs=1) as base:
            actT = base.tile([P, NT, T], F32R, tag="actT")
            q1T = base.tile([P, NT, R], F32R, tag="q1T")
            kT = base.tile([P, NT, T], BF16, tag="kT")
            qb = base.tile([P, NT, R], BF16, tag="qb")
            vi = base.tile([P, NT, NH, P], BF16, tag="vi")
            ctxn = base.tile([P, NT, R], F32R, tag="ctxn")
            resT = base.tile([P, NT, R], F32R, tag="resT")
            slfT = base.tile([P, NT, R], F32R, tag="slfT")
            bqc = base.tile([P, NT, 1], F32, tag="bqc")
            bkc = base.tile([P, NT, 1], F32, tag="bkc")
            bvc = base.tile([P, NT, 1], F32, tag="bvc")
            bdc = base.tile([P, NT, 1], F32, tag="bdc")
            gc = base.tile([P, NT, 1], F32, tag="gc")
            bc = base.tile([P, NT, 1], F32, tag="bc")
            mtc = base.tile([P, NT, 1], F32, tag="mtc")
            msc = base.tile([P, NT, 1], F32, tag="msc")
            ones1 = base.tile([P, 1], F32R, tag="ones1")
            onesr = base.tile([1, P], F32R, tag="onesr")
            epsc = base.tile([P, 1], F32, tag="epsc")
            ident = base.tile([P, P], F32, tag="ident")

            dma = nc.sync.dma_start
            dqp = tc.alloc_tile_pool(name="dqp", bufs=1)
            dqT = dqp.tile([P, NT, R], F32R, tag="dqT")
            dma(out=dqT[:, :, :], in_=dqT_d.ap().bitcast(F32R))
            dma(out=bqc[:, :, :], in_=col_view(bq_d))
            dma(out=bkc[:, :, :], in_=col_view(bk_d))
            dma(out=bvc[:, :, :], in_=col_view(bv_d))
            dma(out=bdc[:, :, :], in_=col_view(bd_d))
            dma(out=gc[:, :, :], in_=col_view(lng_d))
            dma(out=bc[:, :, :], in_=col_view(lnb_d))
            dma(out=mtc[:, :, :], in_=col_view(mt_d))
            dma(out=msc[:, :, :], in_=col_view(ms_d))
            dma(out=ones1[:, :], in_=onesf_d.ap().bitcast(F32R))
            dma(out=onesr[:, :], in_=onesr_d.ap().bitcast(F32R))
            nc.vector.memset(epsc[:, :], EPS)
            onesb = base.tile([P, NH, D], BF16, tag="onesb")
            dma(out=onesb[:, :, :], in_=ones_d.ap().rearrange("p (h c) -> p h c", c=D))
            for st in range(NT):
                nc.vector.tensor_copy(vi[:, st, :, D:P], onesb[:, :, :])
            make_identity(nc, ident[:, :])

            def kproj_unit(src, wchunk, ot, tch, ps, bias, uid):
                """One [o-tile, t-chunk] of a feature-major projection:
                8 accumulating matmuls + biased eviction into kT."""
                tsl = slice(tch * R, (tch + 1) * R)
                pk = ps.tile([P, R], F32, tag="pk", name=f"pk{uid}_{ot}_{tch}")
                for it in range(NT):
                    nc.tensor.matmul(
                        pk[:, :], wchunk[:, it, :], src[:, it, tsl],
                        start=(it == 0), stop=(it == NT - 1))
                nc.vector.tensor_scalar_add(kT[:, ot, tsl], pk[:, :], bias[:, ot, :])

            def vproj_unit(src, wchunk, ot, tch, ps, tps, tpool, uid):
                """One [o-tile, t-chunk] of the v projection: feature-major
                matmuls, then PE-transpose into normal layout in vi."""
                tsl = slice(tch * R, (tch + 1) * R)
                pv = ps.tile([P, R], F32, tag="pk", name=f"pv{uid}_{ot}_{tch}")
                for it in range(NT):
                    nc.tensor.matmul(
                        pv[:, :], wchunk[:, it, :], src[:, it, tsl],
                        start=(it == 0), stop=(it == NT - 1))
                vt = tpool.tile([P, R], F32, tag="vt", name=f"vt{uid}_{ot}_{tch}")
                nc.vector.tensor_scalar_add(vt[:, :], pv[:, :], bvc[:, ot, :])
                for bj in range(R // P):
                    st = tch * (R // P) + bj
                    pt = tps.tile([P, P], F32, tag="pt", name=f"ptv{uid}_{ot}_{st}")
                    nc.tensor.transpose(pt[:, :], vt[:, bj * P:(bj + 1) * P], ident[:, :])
                    nc.vector.tensor_copy(
                        vi[:, st, 2 * ot:2 * ot + 2, 0:D],
                        pt[:, :].rearrange("p (h c) -> p h c", c=D))

            def attention(qsrc, mcol, fillers, uid, min_pair=0):
                """scoresT -> batched exp -> fused PV+denominator -> ctxn.
                fillers: callbacks emitting independent PE work, drained
                across pair iterations (only once j >= min_pair)."""
                fill_i = 0
                with tc.tile_pool(name="prp", bufs=4) as prp, \
                     tc.tile_pool(name="rcp", bufs=3) as rcp, \
                     tc.tile_pool(name="psc", bufs=2, space="PSUM") as psc, \
                     tc.tile_pool(name="pcx", bufs=1 if fillers else 2, space="PSUM") as pcx:
                    for j in range(NH // 2):
                        c0 = pcx.tile([P, R], F32, tag="c0", name=f"c0{uid}_{j}")
                        c1 = pcx.tile([P, R], F32, tag="c1", name=f"c1{uid}_{j}")
                        probs = [None] * NT
                        for st in range(NT + 1):
                            # scores + exp for step st; PV for step st-1 (SW pipeline
                            # so the in-order PE stream never waits on the current exp)
                            if st < NT:
                                ssl = slice(st * P, (st + 1) * P)
                                s01 = psc.tile([P, 2, R], F32, tag="s01", name=f"s{uid}_{j}_{st}")
                                nc.tensor.matmul(
                                    s01[:, 0, :], kT[0:D, j, ssl], qsrc[0:D, j, :])
                                nc.tensor.matmul(
                                    s01[:, 1, :], kT[D:P, j, ssl], qsrc[D:P, j, :])
                                p01 = prp.tile([P, 2, R], BF16, tag="p01", name=f"p{uid}_{j}_{st}")
                                nc.scalar.activation(
                                    p01[:, :, :], s01[:, :, :], AF.Exp,
                                    bias=mcol[:, st, :], scale=0.125)
                                probs[st] = p01
                            if st > 0:
                                pp01 = probs[st - 1]
                                nc.tensor.matmul(
                                    c0[:, :], vi[:, st - 1, 2 * j, :], pp01[:, 0, :],
                                    start=(st == 1), stop=(st == NT))
                                nc.tensor.matmul(
                                    c1[:, :], vi[:, st - 1, 2 * j + 1, :], pp01[:, 1, :],
                                    start=(st == 1), stop=(st == NT))
                        r0 = rcp.tile([D, R], F32, tag="rr", bufs=6, name=f"r0{uid}_{j}")
                        r1 = rcp.tile([D, R], F32, tag="rr", bufs=6, name=f"r1{uid}_{j}")
                        d0 = rcp.tile([D, R], F32, tag="rr", bufs=6, name=f"d0{uid}_{j}")
                        d1 = rcp.tile([D, R], F32, tag="rr", bufs=6, name=f"d1{uid}_{j}")
                        cc0 = rcp.tile([P, R], F32, tag="cc", bufs=2, name=f"cc0{uid}_{j}")
                        cc1 = rcp.tile([P, R], F32, tag="cc", bufs=2, name=f"cc1{uid}_{j}")
                        # one ACT copy frees the PSUM bank immediately; the
                        # reciprocal chain then runs from SBUF (base-0 slice)
                        nc.scalar.copy(cc0[:, :], c0[:, :])
                        nc.scalar.copy(cc1[:, :], c1[:, :])
                        nc.vector.tensor_copy(d0[:, :], cc0[D:P, :])
                        nc.vector.reciprocal_approx_fast(r0[:, :], d0[:, :])
                        nc.vector.tensor_mul(ctxn[0:D, j, :], cc0[0:D, :], r0[:, :])
                        nc.vector.tensor_copy(d1[:, :], cc1[D:P, :])
                        nc.vector.reciprocal_approx_fast(r1[:, :], d1[:, :])
                        nc.vector.tensor_mul(ctxn[D:P, j, :], cc1[0:D, :], r1[:, :])
                        navail = NH // 2 - min_pair
                        while (fillers and j >= min_pair and
                               fill_i < (j - min_pair + 1) * len(fillers) // navail):
                            fillers[fill_i]()
                            fill_i += 1
                    while fill_i < len(fillers):
                        fillers[fill_i]()
                        fill_i += 1

            def proj_ln(resid_src, dst, fillers, uid, partial=None, qcopy=None,
                        hold=0):
                """Out-projection + residual into resT with LN stats fused
                per o-tile; then row stats, broadcast, per-o-tile apply -> dst.
                If partial is given, it holds ht 0..3 of the accumulation and
                only ht 4..7 run here."""
                fill_i = 0
                with tc.tile_pool(name="wdp", bufs=2) as wp, \
                     tc.tile_pool(name="sqp", bufs=2) as sqp, \
                     tc.tile_pool(name="lnp", bufs=1) as lnp, \
                     tc.tile_pool(name="ps3", bufs=2, space="PSUM") as ps, \
                     tc.tile_pool(name="ps4", bufs=1, space="PSUM") as ps4:
                    pmu = ps4.tile([1, R], F32, tag="pmu", name=f"pmu{uid}")
                    psq = ps4.tile([1, R], F32, tag="psq", name=f"psq{uid}")
                    for ot in range(NT):
                        osl = slice(ot * P, (ot + 1) * P)
                        if partial is None:
                            wd_c = wp.tile([P, NT, P], F32R, tag="wd", name=f"wd{uid}_{ot}")
                            dma(out=wd_c[:, :, :], in_=wchunk_view(wdT, ot))
                            pp = ps.tile([P, R], F32, tag="pp", name=f"pp{uid}_{ot}")
                            for ht in range(NT):
                                nc.tensor.matmul(
                                    pp[:, :], wd_c[:, ht, :], ctxn[:, ht, :],
                                    start=(ht == 0), stop=(ht == NT - 1))
                            nc.vector.scalar_tensor_tensor(
                                resT[:, ot, :], pp[:, :], bdc[:, ot, :],
                                resid_src[:, ot, :].bitcast(F32), op0=ALU.add, op1=ALU.add)
                        else:
                            wd_c = wp.tile([P, 4, P], F32R, tag="wd", name=f"wd{uid}_{ot}")
                            dma(out=wd_c[:, :, :], in_=wchunk_view(wdT, ot)[:, 4:NT, :])
                            pp = ps.tile([P, R], F32, tag="pp", name=f"pp{uid}_{ot}")
                            for ht in range(4):
                                nc.tensor.matmul(
                                    pp[:, :], wd_c[:, ht, :], ctxn[:, ht + 4, :],
                                    start=(ht == 0), stop=(ht == 3))
                            tsum = sqp.tile([P, R], F32, tag="tt", name=f"tsum{uid}_{ot}", bufs=2)
                            nc.vector.scalar_tensor_tensor(
                                tsum[:, :], pp[:, :], bdc[:, ot, :],
                                partial[:, ot, :], op0=ALU.add, op1=ALU.add)
                            nc.vector.tensor_add(
                                resT[:, ot, :], tsum[:, :],
                                resid_src[:, ot, :].bitcast(F32))
                        sq = sqp.tile([P, R], F32R, tag="sq", name=f"sq{uid}_{ot}")
                        nc.scalar.square(sq[:, :], resT[:, ot, :].bitcast(F32))
                        nc.tensor.matmul(
                            pmu[:, :], ones1[:, :], resT[:, ot, :],
                            start=(ot == 0), stop=(ot == NT - 1))
                        nc.tensor.matmul(
                            psq[:, :], ones1[:, :], sq[:, :],
                            start=(ot == 0), stop=(ot == NT - 1))
                        early = len(fillers) - hold
                        while fillers and fill_i < (ot + 1) * early // NT:
                            fillers[fill_i]()
                            fill_i += 1
                    mu_r = lnp.tile([1, R], F32R, tag="lnrow", bufs=2, name=f"mu{uid}")
                    nc.scalar.mul(mu_r[:, :], pmu[:, :], 1.0 / H)
                    sq_r = lnp.tile([1, R], F32R, tag="lnrow", bufs=2, name=f"sqr{uid}")
                    nc.scalar.mul(sq_r[:, :], psq[:, :], 1.0 / H)
                    muB = ps4.tile([P, R], F32, tag="pmu", name=f"muBp{uid}")
                    nc.tensor.matmul(muB[:, :], onesr[:, :], mu_r[:, :])
                    sqBp = ps4.tile([P, R], F32, tag="psq", name=f"sqBp{uid}")
                    nc.tensor.matmul(sqBp[:, :], onesr[:, :], sq_r[:, :])
                    msB = sqp.tile([P, R], F32, tag="lnB", name=f"msB{uid}", bufs=2)
                    nc.scalar.square(msB[:, :], muB[:, :])
                    varB = sqp.tile([P, R], F32, tag="lnB", name=f"varB{uid}", bufs=2)
                    nc.vector.tensor_sub(varB[:, :], sqBp[:, :], msB[:, :])
                    sdB = sqp.tile([P, R], F32, tag="lnB", name=f"sdB{uid}", bufs=2)
                    nc.scalar.activation(sdB[:, :], varB[:, :], AF.Sqrt, bias=epsc[:, :])
                    rsB = sqp.tile([P, R], F32, tag="rsB", name=f"rsB{uid}", bufs=1)
                    nc.vector.reciprocal_approx_fast(rsB[:, :], sdB[:, :])
                    for ot in range(NT):
                        t1 = sqp.tile([P, R], F32, tag="tt", name=f"t1{uid}_{ot}", bufs=2)
                        nc.vector.tensor_sub(t1[:, :], resT[:, ot, :].bitcast(F32), muB[:, :])
                        t2 = sqp.tile([P, R], F32, tag="tt", name=f"t2{uid}_{ot}", bufs=2)
                        nc.vector.tensor_mul(t2[:, :], t1[:, :], rsB[:, :])
                        nc.scalar.activation(
                            dst[:, ot, :], t2[:, :], AF.Identity,
                            bias=bc[:, ot, :], scale=gc[:, ot, :])
                        if qcopy is not None:
                            nc.vector.tensor_copy(
                                qcopy[:, ot, :], dst[:, ot, :].bitcast(F32))
                    while fill_i < len(fillers):
                        fillers[fill_i]()
                        fill_i += 1

            # ================== phase 1: decoder projections ==================
            with tc.tile_pool(name="wp1", bufs=3) as wp1, \
                 tc.tile_pool(name="tp1", bufs=3) as tp1, \
                 tc.tile_pool(name="ps1", bufs=3, space="PSUM") as ps1, \
                 tc.tile_pool(name="pst", bufs=3, space="PSUM") as pst:
                for ot in range(NT):
                    wq_c = wp1.tile([P, NT, P], F32R, tag="w", name=f"wq{ot}")
                    dma(out=wq_c[:, :, :], in_=wchunk_view(wqT, ot))
                    pq = ps1.tile([P, R], F32, tag="pk", name=f"pq{ot}")
                    for it in range(NT):
                        nc.tensor.matmul(
                            pq[:, :], wq_c[:, it, :], dqT[:, it, :],
                            start=(it == 0), stop=(it == NT - 1))
                    nc.vector.tensor_scalar_add(q1T[:, ot, :], pq[:, :], bqc[:, ot, :])
                    nc.vector.tensor_copy(qb[:, ot, :], q1T[:, ot, :].bitcast(F32))
                    if ot == 0:
                        dma(out=actT[:, :, :], in_=decT.ap().bitcast(F32R))
                for ot in range(NT):
                    wk_c = wp1.tile([P, NT, P], F32R, tag="w", name=f"wk{ot}")
                    dma(out=wk_c[:, :, :], in_=wchunk_view(wkT, ot))
                    for tch in range(2):
                        kproj_unit(actT, wk_c, ot, tch, ps1, bkc, "a")
                    wv_c = wp1.tile([P, NT, P], F32R, tag="w", name=f"wv{ot}")
                    dma(out=wv_c[:, :, :], in_=wchunk_view(wvT, ot))
                    for tch in range(2):
                        vproj_unit(actT, wv_c, ot, tch, ps1, pst, tp1, "a")

            dqp.release()

            # ============ phase 2: self-attn (+ encoder-k interleaved) ============
            # encT overwrites actT once all phase-1 reads are done.
            dma(out=actT[:, :, :], in_=encT.ap().bitcast(F32R))
            with tc.tile_pool(name="wp2", bufs=2) as wp2, \
                 tc.tile_pool(name="ps2", bufs=2, space="PSUM") as ps2:

                def mk_ek(ot):
                    def f():
                        wk_c = wp2.tile([P, NT, P], F32R, tag="wk2", name=f"wk2_{ot}")
                        dma(out=wk_c[:, :, :], in_=wchunk_view(wkT, ot))
                        for tch in range(2):
                            kproj_unit(actT, wk_c, ot, tch, ps2, bkc, f"b{ot}")
                    return f

                attention(qb, mtc, [mk_ek(ot) for ot in range(NT)], "A")

            # ========= phase 3: out-proj + LN1 (+ encoder-v interleaved) =========
            with tc.tile_pool(name="wp3", bufs=2) as wp3, \
                 tc.tile_pool(name="tp3", bufs=2) as tp3, \
                 tc.tile_pool(name="ps2b", bufs=2, space="PSUM") as ps2b, \
                 tc.tile_pool(name="pstb", bufs=2, space="PSUM") as pstb:
                def mk_ev(ot):
                    def f():
                        wv_c = wp3.tile([P, NT, P], F32R, tag="wv2", name=f"wv2_{ot}")
                        dma(out=wv_c[:, :, :], in_=wchunk_view(wvT, ot))
                        for tch in range(2):
                            vproj_unit(actT, wv_c, ot, tch, ps2b, pstb, tp3, f"b{ot}")
                    return f

                proj_ln(q1T, slfT, [mk_ev(ot) for ot in range(NT)], "A", qcopy=qb)

            # ==================== phase 4: cross-attention ====================
            with tc.tile_pool(name="prt", bufs=1) as prt, \
                 tc.tile_pool(name="wpB", bufs=2) as wpB, \
                 tc.tile_pool(name="psB", bufs=2, space="PSUM") as psB:
                partialA = prt.tile([P, NT, R], F32, tag="partialA")

                def mk_pA(ot):
                    def f():
                        wd_c = wpB.tile([P, 4, P], F32R, tag="wdA", name=f"wdA{ot}")
                        dma(out=wd_c[:, :, :], in_=wchunk_view(wdT, ot)[:, 0:4, :])
                        pp = psB.tile([P, R], F32, tag="ppA", name=f"ppA{ot}")
                        for ht in range(4):
                            nc.tensor.matmul(
                                pp[:, :], wd_c[:, ht, :], ctxn[:, ht, :],
                                start=(ht == 0), stop=(ht == 3))
                        nc.vector.tensor_copy(partialA[:, ot, :], pp[:, :])
                    return f

                attention(qb, msc, [mk_pA(ot) for ot in range(NT)], "B", min_pair=4)
                proj_ln(slfT, slfT, [], "B", partial=partialA)

            # ============== phase 5: store (feature-major; host transposes) =====
            for ot in range(NT):
                dma(out=out_d.ap()[:, ot, :], in_=slfT[:, ot, :].bitcast(F32))

    nc.compile()
    return nc


_NC = None
import ml_dtypes

_ONES = np.ones((P, NH * D), ml_dtypes.bfloat16)
_ONESF = np.ones((P, 1), np.float32)
_ONESR = np.ones((1, P), np.float32)


def make_in_maps(encoder_states, decoder_inputs, src_attention_mask,
                 tgt_attention_mask, Wq, bq, Wk, bk, Wv, bv, Wd, bd, ln_g, ln_b):
    f = np.float32

    def wtile(w):  # [o,i] -> W.T tiled [ot, p, it, c]
        return np.ascontiguousarray(
            np.asarray(w, f).T.reshape(NT, P, NT, P).transpose(2, 1, 0, 3))

    def atile(x):  # [t,i] -> x.T tiled [p, it, t]
        return np.ascontiguousarray(
            np.asarray(x, f).T.reshape(NT, P, -1).transpose(1, 0, 2))

    wqT, wkT, wvT, wdT = wtile(Wq), wtile(Wk), wtile(Wv), wtile(Wd)
    col = lambda x: np.ascontiguousarray(
        np.asarray(x, f).reshape(NT, P).T.reshape(P, NT, 1))
    bq_, bk_, bv_, bd_ = col(bq), col(bk), col(bv), col(bd)
    g_, b_ = col(ln_g), col(ln_b)

    decT_b = [atile(decoder_inputs[b]) for b in range(B)]
    encT_b = [atile(encoder_states[b]) for b in range(B)]
    mt_b = [col(tgt_attention_mask[b, 0, 0, :]) for b in range(B)]
    ms_b = [col(src_attention_mask[b, 0, 0, :]) for b in range(B)]

    in_maps = []
    for c in range(8):
        b, half = c // 2, c % 2
        in_maps.append({
            "decT": decT_b[b],
            "decqT": np.ascontiguousarray(decT_b[b][:, :, half * R:(half + 1) * R]),
            "encT": encT_b[b],
            "wqT": wqT, "wkT": wkT, "wvT": wvT, "wdT": wdT,
            "bq": bq_, "bk": bk_, "bv": bv_, "bd": bd_,
            "lng": g_, "lnb": b_,
            "mt": mt_b[b], "ms": ms_b[b],
            "onesc": _ONES, "onesf": _ONESF, "onesr": _ONESR,
        })
    return in_maps


def kernel(**inputs):
    global _NC
    if _NC is None:
        _NC = build_kernel()
    nc = _NC
    in_maps = make_in_maps(**inputs)
    res = run_bass_kernel_spmd(nc, in_maps, core_ids=list(range(8)))
    out = np.empty((B, T, H), np.float32)
    for c in range(8):
        b, half = c // 2, c % 2
        buf = res.results[c]["out"]  # [p, ot, t]
        out[b, half * R:(half + 1) * R, :] = (
            buf.transpose(2, 1, 0).reshape(R, H))
    return out
